# revision 7
# baseline (speedup 1.0000x reference)
"""CTC greedy decode kernel for Trainium2 (Bass/Tile), 8-core data-parallel.

Problem: log_probs [32, 4096, 1025] f32, input_lengths [32] i64 ->
  preds    [32, 4096] int32  (per-frame argmax)
  keep     [32, 4096] bool   (non-blank & != prev & t < len)
  max_logp [32, 4096] f32    (value at argmax)

Sharding: batch dim across 8 cores (4 utterances each). Per core:
16384 frames x 1025 vocab. Frames ride the SBUF partition dim (128
frames/tile, 128 tiles); vocab rides the free dim.

Argmax without a second DVE scan (the two-pass reduce+max_index version
is vector-bound at ~273us; DMA of the 67MB/core input is ~187us):

  1. DVE tensor_tensor_scan (op0=op1=max, data1=data0) computes the
     running prefix-max P_v of each frame in ONE pass. Its last element
     is the frame max m (exact f32, also the max_logp output).
  2. The Activation engine computes Sign(-P_v + m) -- 1 where P_v < m,
     0 where P_v == m -- and its accum_out sums the pass: the count of
     prefix positions strictly below the max IS the argmax index, with
     exact first-occurrence tie-breaking (jnp.argmax semantics) for any
     input, duplicates included.

So DVE does one 1.04ns/elem pass (~150us), ACT one 0.83ns/elem pass
(~137us), and the ~187us HBM stream is the critical path. The CTC
collapse mask is a handful of small [128,128] grid ops; grid columns
0:96 are finalized and stored while the tail tiles stream, so the
post-DMA tail is one tile's scan+sign plus the last prev-shift/store.
"""

import numpy as np

import concourse.bacc as bacc
import concourse.mybir as mybir
from concourse.tile import TileContext
from concourse.bass_utils import run_bass_kernel_spmd

B, T, V = 32, 4096, 1025
BLANK = 1024
NCORES = 8
BLOC = B // NCORES        # utterances per core
F = BLOC * T              # frames per core
P = 128                   # partitions
NT = F // P               # tiles per core (128)
CPU = T // P              # columns per utterance (32)
NGRP = NT // 4            # 4-tile groups
NFULL = NGRP - 1          # full groups; last 4 tiles load per-tile
CSPLIT = 96               # grid column where early/late epilogue splits

_CACHE = {}


def _build_program():
    nc = bacc.Bacc(None, target_bir_lowering=False)
    f32 = mybir.dt.float32
    i32 = mybir.dt.int32
    lp = nc.dram_tensor("lp", [F, V], f32, kind="ExternalInput")
    valid = nc.dram_tensor("valid", [P, NT], f32, kind="ExternalInput")
    preds_o = nc.dram_tensor("preds", [P, NT], i32, kind="ExternalOutput")
    keep_o = nc.dram_tensor("keep", [P, NT], i32, kind="ExternalOutput")
    mlp_o = nc.dram_tensor("maxlp", [P, NT], f32, kind="ExternalOutput")

    # frame f = n*128 + p  ->  [p, n, v]
    lp_r = lp.rearrange("(n p) v -> p n v", p=P)
    SIGN = mybir.ActivationFunctionType.Sign

    with TileContext(nc) as tc:
        with (
            tc.tile_pool(name="loads", bufs=4) as loads,
            tc.tile_pool(name="pms", bufs=4) as pms,
            tc.tile_pool(name="sgs", bufs=3) as sgs,
            tc.tile_pool(name="persist", bufs=1) as pp,
        ):
            first = loads.tile([P, 4, V], f32, tag="big")
            nc.sync.dma_start(out=first[:], in_=lp_r[:, 0:4, :])

            idxf = pp.tile([P, NT], f32)     # argmax index (exact int in f32)
            gmax = pp.tile([P, NT], f32)     # frame max (max_logp output)
            prev = pp.tile([P, NT], f32)
            validt = pp.tile([P, NT], f32)
            k1 = pp.tile([P, NT], f32)
            kp = pp.tile([P, NT], f32)
            preds_i = pp.tile([P, NT], i32)
            keep_i = pp.tile([P, NT], i32)

            nc.sync.dma_start(out=validt[:], in_=valid[:])

            def tile_pass(src2d, pm2d, col):
                # one frame-tile: prefix-max scan, then Sign+accumulate
                nc.vector.tensor_tensor_scan(
                    out=pm2d, data0=src2d, data1=src2d,
                    initial=-3.0e38,
                    op0=mybir.AluOpType.max, op1=mybir.AluOpType.max,
                )
                sg = sgs.tile([P, V], f32, tag="sg")
                nc.scalar.activation(
                    out=sg[:], in_=pm2d, func=SIGN,
                    bias=pm2d[:, V - 1 : V], scale=-1.0,
                    accum_out=idxf[:, col : col + 1],
                )

            def finalize(c0, c1):
                # grid epilogue for columns [c0, c1): prev-shift, CTC mask,
                # int convert, store. Wrap row reads column c0-1 from idxf
                # when c0 > 0 (that column is final before this runs).
                # Emitted only after every load is queued on SP, so its
                # sem-waits never head-of-line-block the load stream.
                nc.vector.tensor_copy(
                    out=preds_i[:, c0:c1], in_=idxf[:, c0:c1]
                )
                nc.sync.dma_start(out=preds_o[:, c0:c1], in_=preds_i[:, c0:c1])
                nc.sync.dma_start(out=mlp_o[:, c0:c1], in_=gmax[:, c0:c1])
                nc.sync.dma_start(
                    out=prev[1:P, c0:c1], in_=idxf[0 : P - 1, c0:c1]
                )
                w0 = max(c0, 1)
                nc.sync.dma_start(
                    out=prev[0:1, w0:c1], in_=idxf[P - 1 : P, w0 - 1 : c1 - 1]
                )
                sent = prev.rearrange("p (u c) -> p u c", c=CPU)
                for u in range(BLOC):
                    if c0 <= u * CPU < c1:
                        nc.gpsimd.memset(sent[0:1, u : u + 1, 0:1], -1.0)
                nc.vector.tensor_scalar(
                    out=k1[:, c0:c1], in0=idxf[:, c0:c1],
                    scalar1=float(BLANK), scalar2=None,
                    op0=mybir.AluOpType.not_equal,
                )
                nc.vector.tensor_tensor(
                    out=k1[:, c0:c1], in0=k1[:, c0:c1], in1=validt[:, c0:c1],
                    op=mybir.AluOpType.mult,
                )
                nc.vector.tensor_tensor(
                    out=kp[:, c0:c1], in0=idxf[:, c0:c1], in1=prev[:, c0:c1],
                    op=mybir.AluOpType.not_equal,
                )
                nc.vector.tensor_tensor(
                    out=keep_i[:, c0:c1], in0=kp[:, c0:c1], in1=k1[:, c0:c1],
                    op=mybir.AluOpType.mult,
                )
                nc.sync.dma_start(out=keep_o[:, c0:c1], in_=keep_i[:, c0:c1])

            for g in range(NFULL):
                i0 = g * 4
                if g == 0:
                    big = first
                else:
                    big = loads.tile([P, 4, V], f32, tag="big")
                    nc.sync.dma_start(out=big[:], in_=lp_r[:, i0 : i0 + 4, :])
                pm = pms.tile([P, 4, V], f32, tag="pm")
                for i in range(4):
                    tile_pass(big[:, i, :], pm[:, i, :], i0 + i)
                nc.gpsimd.tensor_copy(
                    out=gmax[:, i0 : i0 + 4], in_=pm[:, :, V - 1]
                )

            for k, t in enumerate(range(NFULL * 4, NT)):
                bt = loads.tile([P, 1, V], f32, tag="tail")
                nc.sync.dma_start(out=bt[:], in_=lp_r[:, t : t + 1, :])
                pmt = pms.tile([P, 1, V], f32, tag="pmt")
                tile_pass(bt[:, 0, :], pmt[:, 0, :], t)
                nc.gpsimd.tensor_copy(
                    out=gmax[:, t : t + 1], in_=pmt[:, 0, V - 1 : V]
                )
            finalize(0, CSPLIT)
            finalize(CSPLIT, NT)
    nc.compile()
    return nc


def _host_inputs(log_probs, input_lengths):
    log_probs = np.ascontiguousarray(np.asarray(log_probs, dtype=np.float32))
    lens = np.asarray(input_lengths).astype(np.int64)
    cols = np.arange(NT)
    # valid mask: frame t = (c%32)*128 + p < len(utterance c//32)
    tvals = (cols % CPU)[None, :] * P + np.arange(P)[:, None]
    in_maps = []
    for c in range(NCORES):
        lp_c = log_probs[c * BLOC : (c + 1) * BLOC].reshape(F, V)
        lens_c = lens[c * BLOC : (c + 1) * BLOC]
        vmask = (tvals < lens_c[cols // CPU][None, :]).astype(np.float32)
        in_maps.append({"lp": lp_c, "valid": np.ascontiguousarray(vmask)})
    return in_maps


def _grid_to_bt(arr):
    # arr [P, NT]: value for frame t=(col%32)*128+p of utterance col//32
    return arr.reshape(P, BLOC, CPU).transpose(1, 2, 0).reshape(BLOC, T)


def kernel(log_probs, input_lengths, **_kw):
    if "nc" not in _CACHE:
        _CACHE["nc"] = _build_program()
    nc = _CACHE["nc"]
    in_maps = _host_inputs(log_probs, input_lengths)
    res = run_bass_kernel_spmd(nc, in_maps, core_ids=list(range(NCORES)))
    preds = np.empty((B, T), dtype=np.int32)
    keep = np.empty((B, T), dtype=bool)
    max_logp = np.empty((B, T), dtype=np.float32)
    for c, r in enumerate(res.results):
        sl = slice(c * BLOC, (c + 1) * BLOC)
        preds[sl] = _grid_to_bt(r["preds"])
        keep[sl] = _grid_to_bt(r["keep"]).astype(bool)
        max_logp[sl] = _grid_to_bt(r["maxlp"])
    return preds, keep, max_logp
